# revision 28
# baseline (speedup 1.0000x reference)
"""Evo2 attention (B=2, S=2048, HID=2048, NH=16, HD=128) on 8 trn2 NeuronCores.

Sharding: core c handles batch b=c//4 and heads 4*(c%4)..4*(c%4)+3.
Megatron-style: q/k/v projections column-parallel, o_proj row-parallel with the
4-way partial sum done on host during unshard.

Per-core kernel layout (everything transposed so no on-chip transposes needed):
  hsT [hid, tok] -> qT,kT [hd, tok] (RoPE fused into PSUM eviction, rotate-half
  basis obtained by de-interleaving W rows on host), v [tok, hd].
  scoresT[k, q] = kT_blk vs qT matmul; softmax over k (= partitions) with a
  fixed shift instead of a max; denominators via ones-vector PE reduction and
  a K=1 matmul broadcast; PV gives attnT [hd, q]; o_projT partial [o, q].

Optimizations vs the fp32r baseline (429us -> 331us -> ~318us):
  - all matmul operands bf16 (PSUM accumulation stays fp32): halves DMA+SBUF,
    enables FWL weight loads; output partials bf16, summed fp32 on host
  - phase A head kill: q and k passes are kc-OUTER across 4 psum banks each
    with 2-kc weight piece tiles (2KB DMA lines -- 1-kc pieces halve the DMA
    rate) and consumption-ordered DMA interleave across BOTH rings (hs on
    sync, wq on gpsimd), so the first real matmul gates on ~768KB (not 4MB)
    and chunk 0 streams DMA-paced from ~10us. Warmup = many SMALL dummy
    matmuls (N=128/64): the preamble end varies +-2.5us run to run, so when
    data arrives mid-warmup the queued dummies must drain in ~100ns steps.
  - early dummy exp (ACT table-set preload off the critical path)
  - causal diagonal blocks column-trimmed: fully-masked leading q-columns of
    each [128k x 512q] tile are never computed (score/exp/den/PV all skip)
  - causal masking via a [128,128] bf16 triangle MULTIPLY on probs post-exp
    (masked probs become exact zeros) instead of a [128,512] f32 mask add
    pre-exp: ~25us less DVE work and a shorter score->exp chain. Generic
    (non-causal-triangle) masks keep the old add-a-mask-tile path.
  - reciprocal -> single-pass reciprocal_approx_fast (~5x faster DVE op)
  - denominators: off>0 probs tiles zero-padded by gpsimd memsets so every
    (h,qc) collapses via a rolling binary tree of bf16 DVE adds into ONE
    N=512 PE reduction stream (den matmuls were ~11us of PE before); for the
    last head the final two blocks feed accumulating den matmuls directly so
    no DVE adds sit on the closing attn chain. (Routing the level-0 adds to
    gpsimd tensor_add was tried and cost +40us -- gpsimd elementwise is far
    slower than its memsets; don't.)
  - qc=0 (no o-proj filler available) runs its heads as interleaved PAIRS
    for chain-level parallelism; elsewhere o-projection of the previous
    q-chunk is interleaved INTO the score/PV loop at block granularity as PE
    filler for the exp-chain waits, with 2 iblks held back past the last
    head's j-loop to cover its den/reciprocal/attn drain
  - dependency-granularity fixes: per-(head,chunk) kt tiles, perm-scoped mask
    pool
"""
import os
import sys
import numpy as np

for _p in ("/opt/trn_rl_repo",):
    if os.path.isdir(_p) and _p not in sys.path:
        sys.path.insert(0, _p)

import ml_dtypes

BF16 = ml_dtypes.bfloat16
B, S, HID, NH = 2, 2048, 2048, 16
HD = HID // NH            # 128
HPC = 4                   # heads per core
NCORES = 8
BASE = 10000.0
SCALE = 1.0 / np.sqrt(HD).astype(np.float32)
SHIFT = 25.0              # fixed softmax shift (replaces per-row max)
NEG_INF_THRESH = -1e8

_PROGRAM_CACHE = {}


def _rope_tables():
    """cos/±sin tables [HD, S] in the de-interleaved (rotate-half) basis.

    Reference pairs dims (2m, 2m+1) with angle theta_m(s) = s * inv_freq[f(m)],
    f(m) = 2m for m<32 else 2m-64 (from emb[:, ::2] of concat([freqs, freqs])).
    After de-interleave perm [0,2,..126,1,3,..127]: new dim m<64 is old 2m,
    new dim 64+m is old 2m+1.
      out[m]    = x[m] cos_m - x[64+m] sin_m
      out[64+m] = x[m] sin_m + x[64+m] cos_m
    """
    inv_freq = BASE ** (-np.arange(0, HD, 2, dtype=np.float64) / HD)  # [64]
    m = np.arange(64)
    fmap = np.where(m < 32, 2 * m, 2 * m - 64)
    t = np.arange(S, dtype=np.float64)
    theta = t[None, :] * inv_freq[fmap][:, None]          # [64, S]
    cos = np.cos(theta)
    sin = np.sin(theta)
    cosT = np.concatenate([cos, cos], axis=0).astype(np.float32)      # [128, S]
    # row d holds the factor applied to SOURCE half d (dest = other half):
    # src lo -> dst hi uses +sin; src hi -> dst lo uses -sin
    ssinT = np.concatenate([sin, -sin], axis=0).astype(np.float32)    # [128, S]
    return cosT, ssinT


def _mask_plan(mask2d):
    """Classify [128k x 512q] blocks of mask^T. Returns (plan, tiles).

    plan[qc] = list of (kb, mask_tile_idx_or_None, col_off, tri); fully-masked
    blocks skipped; col_off = count of leading fully-masked q-columns (those
    columns are skipped entirely in score/exp/den/PV). tri=True means the
    block's only masking is a causal triangle in its leading 128 columns
    (after the off trim) -- handled by a post-exp triangle multiply, no mask
    tile needed. tiles: deduped f32 [128, 512] mask^T blocks prescaled by
    sqrt(HD) for the generic fallback path.
    """
    maskT = np.ascontiguousarray(mask2d.T)  # [k, q]
    plan = []
    tiles = []
    seen = {}
    for qc in range(S // 512):
        row = []
        for kb in range(S // 128):
            sub = maskT[kb * 128:(kb + 1) * 128, qc * 512:(qc + 1) * 512]
            masked = sub <= NEG_INF_THRESH
            if masked.all():
                continue
            colmasked = masked.all(axis=0)  # [512]
            off = 0
            while off < 512 and colmasked[off]:
                off += 1
            if (sub[:, off:] == 0.0).all():
                row.append((kb, None, off, False))
                continue
            w = min(128, 512 - off)
            tri_pat = np.tril(np.ones((128, 128), dtype=bool), -1)[:, :w]
            sub_w = sub[:, off:off + w]
            if (np.array_equal(masked[:, off:off + w], tri_pat)
                    and (sub_w[~tri_pat] == 0.0).all()
                    and (sub[:, off + w:] == 0.0).all()):
                row.append((kb, None, off, True))
                continue
            pre = np.ascontiguousarray(sub * np.float32(np.sqrt(HD)))
            key = pre.tobytes()
            idx = seen.get(key)
            if idx is None:
                idx = len(tiles)
                seen[key] = idx
                tiles.append(pre)
            row.append((kb, idx, off, False))
        plan.append(row)
    return plan, tiles


def _build_program(plan, nm, preload_masks, any_tri):
    import contextlib
    import concourse.bacc as bacc
    import concourse.tile as tile
    from concourse import mybir

    f32 = mybir.dt.float32
    f32r = mybir.dt.float32r
    bf16 = mybir.dt.bfloat16
    nc = bacc.Bacc(None, target_bir_lowering=False)

    # host-pre-tiled inputs: last axis group per partition is contiguous
    hs_d = nc.dram_tensor("hs_t", [4, 128, 16, 512], bf16, kind="ExternalInput")
    wq_d = nc.dram_tensor("wq_t", [128, 16, 4, 128], bf16, kind="ExternalInput")
    wk_d = nc.dram_tensor("wk_t", [128, 16, 4, 128], bf16, kind="ExternalInput")
    wv_d = nc.dram_tensor("wv_t", [128, 16, 512], bf16, kind="ExternalInput")
    wo_d = nc.dram_tensor("wo_t", [128, 4, 16, 128], bf16, kind="ExternalInput")
    cos_d = nc.dram_tensor("cosT", [128, S], bf16, kind="ExternalInput")
    sin_d = nc.dram_tensor("ssinT", [128, S], bf16, kind="ExternalInput")
    if any_tri:
        tri_d = nc.dram_tensor("tri_t", [128, 128], bf16, kind="ExternalInput")
    if nm:
        mask_d = nc.dram_tensor("maskt", [nm, 128, 512], bf16, kind="ExternalInput")
    o_d = nc.dram_tensor("oT_t", [4, 16, 128, 512], bf16, kind="ExternalOutput")

    Exp = mybir.ActivationFunctionType.Exp

    with tile.TileContext(nc) as tc:
        with contextlib.ExitStack() as perm:
            kt_pool = perm.enter_context(tc.tile_pool(name="kt", bufs=16))
            qa_pool = perm.enter_context(tc.tile_pool(name="qa", bufs=17))
            cst = perm.enter_context(tc.tile_pool(name="cst", bufs=1))
            tmp_pool = perm.enter_context(tc.tile_pool(name="tmp", bufs=2))
            wo_pool = perm.enter_context(tc.tile_pool(name="wo", bufs=1))

            wup_st = cst.tile([128, 512], f32, tag="wu0")
            nc.vector.memset(wup_st, 0.0)
            wup = cst.tile([128, 512], bf16, tag="wu1")
            nc.vector.tensor_copy(out=wup, in_=wup_st)

            onesq_st = cst.tile([128, 128], f32, tag="o3")
            nc.vector.memset(onesq_st, 1.0)
            onesq_r = cst.tile([128, 128], bf16, tag="o4")
            nc.vector.tensor_copy(out=onesq_r, in_=onesq_st)
            ones_r1 = cst.tile([128, 128], f32r, tag="o5")
            nc.vector.tensor_copy(out=ones_r1[0:1, :], in_=onesq_st[0:1, :])
            shiftb = cst.tile([128, 1], f32, tag="sh")
            nc.vector.memset(shiftb, -SHIFT)
            # preload the exp ACT table set now; otherwise the first phase-B
            # exp pays the ~2.7us ACT_TABLE_LOAD + DRAIN on the critical path
            expwarm = cst.tile([128, 1], f32, tag="ew")
            nc.scalar.activation(expwarm[:], shiftb[:], Exp)

            tri_sb = None
            if any_tri:
                tri_sb = cst.tile([128, 128], bf16, tag="tri")
                nc.gpsimd.dma_start(out=tri_sb, in_=tri_d[:, :])

            # per-(head, chunk) kt tiles: tile-granular dependency tracking
            # means one big [128,S] tile would gate phase B's first score
            # matmul on the LAST chunk's rope writes
            kt = [[kt_pool.tile([128, 512], bf16, tag="kt", name=f"kt{i}_{c}")
                   for c in range(4)] for i in range(HPC)]

            # mask tiles live in a perm-scoped pool: allocated inside phase
            # B's stack their SBUF range overlaps phase A's pools, which makes
            # the DMA wait for ALL phase-A matmuls and puts ~5us of mask
            # latency on the first exp chain.
            mask_sb = None
            if nm and preload_masks:
                mkp = perm.enter_context(tc.tile_pool(name="mk", bufs=nm))
                mask_sb = [mkp.tile([128, 512], bf16, tag="mk", name=f"mk{i}")
                           for i in range(nm)]
            v_tiles = [None] * 16
            q_tiles = {}
            attn_tiles = {}

            def rope_evict(ps, dst, cos_sb, sin_sb):
                nc.vector.tensor_mul(dst[0:64, :], ps[64:128, :], sin_sb[64:128, :])
                nc.vector.tensor_mul(dst[64:128, :], ps[0:64, :], sin_sb[0:64, :])
                t = tmp_pool.tile([128, 512], f32, tag="ropetmp")
                nc.vector.tensor_mul(t[:], ps[:], cos_sb[:, :])
                nc.vector.tensor_add(dst[:, :], dst[:, :], t[:])

            # ---- Phase A: projections. One hs residency per 512-token
            # chunk feeds the q, k AND v passes. q and k passes are kc-outer
            # (4 parallel psum accumulation groups) so the instruction stream
            # consumes weight pieces in DMA arrival order -- chunk 0 runs
            # DMA-paced with no dead head.
            with contextlib.ExitStack() as actx:
                hs_pool = actx.enter_context(tc.tile_pool(name="hs", bufs=8))
                tabp = actx.enter_context(tc.tile_pool(name="tab", bufs=4))
                w_pool = actx.enter_context(tc.tile_pool(name="w", bufs=1))
                wqp_pool = actx.enter_context(tc.tile_pool(name="wqp", bufs=8))
                wkp_pool = actx.enter_context(tc.tile_pool(name="wkp", bufs=8))
                psQ = actx.enter_context(tc.tile_pool(name="psQ", bufs=4, space="PSUM"))
                psK = actx.enter_context(tc.tile_pool(name="psK", bufs=4, space="PSUM"))
                v_pool = perm.enter_context(tc.tile_pool(name="v", bufs=16, side="right"))

                def load_hs(c):
                    ts = []
                    for i in range(4):
                        ht = hs_pool.tile([128, 4, 512], bf16, tag="hs",
                                          name=f"hs{c}_{i}")
                        nc.sync.dma_start(out=ht, in_=hs_d[c, :, 4 * i:4 * i + 4, :])
                        ts.append(ht)
                    return ts

                # ~5us of SMALL dummy matmuls: HAM un-throttles while the
                # first DMA pieces land. Small N so that when data arrives
                # mid-warmup, the queued dummies drain in ~100ns steps (the
                # preamble end time varies +-2.5us run to run, so the warmup
                # cannot be tuned to a fixed matmul count at N=512).
                ps_wu = psK.tile([128, 512], f32, tag="k", name="pswu")
                for i in range(40):
                    nc.tensor.matmul(ps_wu[:, 0:128], wup[:, 0:128],
                                     wup[:, 0:128], start=True, stop=True)
                for i in range(24):
                    nc.tensor.matmul(ps_wu[:, 0:64], wup[:, 0:128],
                                     wup[:, 0:64], start=True, stop=True)

                wv_all = w_pool.tile([128, 16, 512], bf16, tag="wv", name="wvall")
                wo_all = wo_pool.tile([128, 4, 16, 128], bf16, tag="wo", name="woall")
                # consumption-ordered interleave: the kc-outer q-pass needs
                # only (hs piece kc//4, wq piece kc//2) per kc step, so the
                # first real matmul gates on ~768KB and the pass streams
                # DMA-paced. Pieces are 2-kc (weights) / 4-kc (hs) so the
                # per-partition DMA lines stay 2KB/4KB (1-kc pieces halve the
                # achieved DMA rate).
                wqp = [wqp_pool.tile([128, 2, 4, 128], bf16, tag="wqp",
                                     name=f"wqp{p}") for p in range(8)]
                wkp = [wkp_pool.tile([128, 2, 4, 128], bf16, tag="wkp",
                                     name=f"wkp{p}") for p in range(8)]
                # wq pieces go on the gpsimd DMA ring: both rings fill in
                # parallel, so the first real matmul's gate (hs piece 0 +
                # wq piece 0) lands ~2us sooner than on one serial ring
                hs_cur = []
                for i in range(4):
                    ht = hs_pool.tile([128, 4, 512], bf16, tag="hs", name=f"hs0_{i}")
                    nc.sync.dma_start(out=ht, in_=hs_d[0, :, 4 * i:4 * i + 4, :])
                    hs_cur.append(ht)
                    nc.gpsimd.dma_start(out=wqp[2 * i], in_=wq_d[:, 4 * i:4 * i + 2, :, :])
                    nc.gpsimd.dma_start(out=wqp[2 * i + 1], in_=wq_d[:, 4 * i + 2:4 * i + 4, :, :])
                for p in range(8):
                    nc.sync.dma_start(out=wkp[p], in_=wk_d[:, 2 * p:2 * p + 2, :, :])
                for p in range(4):
                    nc.sync.dma_start(out=wv_all[:, p * 4:(p + 1) * 4, :],
                                      in_=wv_d[:, p * 4:(p + 1) * 4, :])
                for p in range(4):
                    nc.sync.dma_start(out=wo_all[:, p, :, :],
                                      in_=wo_d[:, p, :, :])
                if mask_sb is not None:
                    for i in range(nm):
                        nc.sync.dma_start(out=mask_sb[i], in_=mask_d[i, :, :])
                for c in range(4):
                    hs_nxt = load_hs(c + 1) if c < 3 else None
                    cos_sb = tabp.tile([128, 512], bf16, tag="cos")
                    nc.gpsimd.dma_start(out=cos_sb, in_=cos_d[:, c * 512:(c + 1) * 512])
                    sin_sb = tabp.tile([128, 512], bf16, tag="sin")
                    nc.gpsimd.dma_start(out=sin_sb, in_=sin_d[:, c * 512:(c + 1) * 512])
                    # q-pass, kc-outer across 4 psum banks
                    psq = [psQ.tile([128, 512], f32, tag="q", name=f"psq{c}_{d}")
                           for d in range(4)]
                    for kc in range(16):
                        for dblk in range(4):
                            nc.tensor.matmul(
                                psq[dblk][:], wqp[kc // 2][:, kc % 2, dblk, :],
                                hs_cur[kc // 4][:, kc % 4, :],
                                start=(kc == 0), stop=(kc == 15))
                    for dblk in range(4):
                        q = qa_pool.tile([128, 512], bf16, tag="qa")
                        rope_evict(psq[dblk], q, cos_sb, sin_sb)
                        q_tiles[(dblk, c)] = q
                    # k-pass, kc-outer (separate banks: no wait on q evicts)
                    psk = [psK.tile([128, 512], f32, tag="k", name=f"psk{c}_{d}")
                           for d in range(4)]
                    for kc in range(16):
                        for dblk in range(4):
                            nc.tensor.matmul(
                                psk[dblk][:], wkp[kc // 2][:, kc % 2, dblk, :],
                                hs_cur[kc // 4][:, kc % 4, :],
                                start=(kc == 0), stop=(kc == 15))
                    for dblk in range(4):
                        rope_evict(psk[dblk], kt[dblk][c], cos_sb, sin_sb)
                    # v pass (vblk-outer, kc-inner; reuses the q psum banks
                    # after their rope evictions, evictions stagger onto ACT)
                    for vblk in range(4):
                        psv = psQ.tile([128, 512], f32, tag="q",
                                       name=f"psv{c}_{vblk}")
                        for kc in range(16):
                            nc.tensor.matmul(
                                psv[:],
                                hs_cur[kc // 4][:, kc % 4, vblk * 128:(vblk + 1) * 128],
                                wv_all[:, kc, :], start=(kc == 0), stop=(kc == 15))
                        vt = v_pool.tile([128, 512], bf16, tag="v", name=f"v{c}_{vblk}")
                        nc.scalar.copy(out=vt[:], in_=psv[:])
                        v_tiles[c * 4 + vblk] = vt
                    hs_cur = hs_nxt

            # ---------------- Phase B + C ------------------------------------
            with contextlib.ExitStack() as bctx:
                probs_pool = bctx.enter_context(tc.tile_pool(name="pr", bufs=8))
                dsum_pool = bctx.enter_context(tc.tile_pool(name="ds", bufs=12))
                smx_pool = bctx.enter_context(tc.tile_pool(name="sm", bufs=2))
                den_pool = bctx.enter_context(tc.tile_pool(name="dn", bufs=4))
                rcb_pool = bctx.enter_context(tc.tile_pool(name="rcb", bufs=3))
                outb_pool = bctx.enter_context(tc.tile_pool(name="ob", bufs=6))
                # PSUM budget (8 banks): scores "s" 2x1 + "s2" 1x2 = 4, att
                # 2, o-proj/den "o" 2. Den matmuls allocate from the psC "o"
                # rotation -- that freed bank is what pays for the 2-bank
                # paired-exp score tiles.
                psB_s = bctx.enter_context(tc.tile_pool(name="psBs", bufs=2, space="PSUM"))
                psB_a = bctx.enter_context(tc.tile_pool(name="psBa", bufs=2, space="PSUM"))
                psC = bctx.enter_context(tc.tile_pool(name="psC", bufs=2, space="PSUM"))
                if nm and not preload_masks:
                    mp = bctx.enter_context(tc.tile_pool(name="mk", bufs=8))

                def emit_tail(state):
                    h, qc, ps_att, den_sb = state
                    rcb = rcb_pool.tile([128, 512], f32, tag="rcb")
                    nc.vector.reciprocal_approx_fast(out=rcb[:], in_=den_sb[:])
                    at = qa_pool.tile([128, 512], bf16, tag="qa")
                    nc.vector.tensor_mul(at[:], ps_att[:], rcb[:])
                    attn_tiles[(h, qc)] = at

                def emit_c_chunk(qc, iblks):
                    for iblk in iblks:
                        ps_o = psC.tile([128, 512], f32, tag="o")
                        for jc in range(4):
                            nc.tensor.matmul(ps_o[:], wo_all[:, jc, iblk, :],
                                             attn_tiles[(jc, qc)][:],
                                             start=(jc == 0), stop=(jc == 3))
                        ob = outb_pool.tile([128, 512], bf16, tag="ob")
                        if iblk % 8 < 6:
                            nc.vector.tensor_copy(out=ob[:], in_=ps_o[:])
                        else:
                            nc.scalar.copy(out=ob[:], in_=ps_o[:])
                        nc.sync.dma_start(out=o_d[qc, iblk, :, :], in_=ob[:])

                def den_push(hst, t):
                    # den: every probs tile is full-width valid (off>0 blocks
                    # are zero-padded by an idle-engine gpsimd memset), so the
                    # whole (h,qc) collapses via a rolling binary tree of bf16
                    # DVE adds into ONE N=512 PE reduction stream (pushed 2
                    # blocks behind the score loop, off the PE critical path)
                    levels = hst["levels"]
                    lvl = 0
                    cur = t
                    while levels[lvl] is not None:
                        s = dsum_pool.tile([128, 512], bf16, tag="ds")
                        nc.vector.tensor_add(s[:], levels[lvl][:], cur[:])
                        levels[lvl] = None
                        cur = s
                        lvl += 1
                    levels[lvl] = cur

                tails = []
                for qc in range(4):
                    kbs = plan[qc]
                    nkb = len(kbs)
                    # qc=0 has no o-proj filler available (no previous qc), so
                    # its short 4-block chains run as interleaved HEAD PAIRS
                    # for chain-level parallelism instead
                    groups = [(0, 1), (2, 3)] if qc == 0 else \
                             [(h,) for h in range(HPC)]
                    for grp in groups:
                        # flush deferred tails first: the in-loop o-proj
                        # filler needs the previous qc's LAST attn tile
                        while tails:
                            emit_tail(tails.pop(0))
                        last33 = (qc == 3 and grp[0] == 3)
                        st = {}
                        for h in grp:
                            st[h] = {
                                "att": psB_a.tile([128, 512], f32, tag="att",
                                                  name=f"att{qc}_{h}"),
                                "prs": [None] * nkb,
                                "levels": [None] * 6,
                                "pend": None,   # prev item's PV specs
                                "pend2": None,  # item-2's specs (den lag)
                                "npv": 0,
                            }
                        # item schedule: runs of off==0 blocks PAIR into one
                        # [128,2,512] 2-bank psum tile so ONE exp covers both
                        # (the ~275ns fixed ACT cost per activation was ~44us
                        # across 160 exps); diag/odd singles are interleaved
                        # between pairs so consecutive pairs never wait on the
                        # single s2 slot's previous exp. qc=0 (head-pair
                        # groups) and the last head (den-stream tail needs
                        # kbs order) stay all-single.
                        if qc == 0 or last33:
                            sched = [("s", j) for j in range(nkb)]
                        else:
                            off0_js = [j for j, e in enumerate(kbs) if e[2] == 0]
                            diag_js = [j for j, e in enumerate(kbs) if e[2] != 0]
                            pi = [(off0_js[k], off0_js[k + 1])
                                  for k in range(0, len(off0_js) - 1, 2)]
                            si = ([off0_js[-1]] if len(off0_js) % 2 else []) \
                                + diag_js
                            sched = []
                            while pi or si:
                                if pi:
                                    sched.append(("p", pi.pop(0)))
                                if si:
                                    sched.append(("s", si.pop(0)))
                        nit = len(sched)
                        # o-proj filler positions: prev-qc iblks emitted inside
                        # the item loop (phase B is exp-chain paced; the
                        # independent o-proj matmuls absorb the PE waits). For
                        # the very last head, 2 iblks are held back to fill
                        # the PE gap while its den/tail chain drains.
                        post_fill = []
                        fjs = {}
                        if qc > 0:
                            base = 4 * grp[0]
                            if last33:
                                fjs = {nit // 2 - 1: [base], nit - 1: [base + 1]}
                                post_fill = [base + 2, base + 3]
                            else:
                                fjs = {(i + 1) * nit // 4 - 1: [base + i]
                                       for i in range(4)}

                        def emit_single(h, hst, j):
                            kb, mi, off, tri = kbs[j]
                            ps_s = psB_s.tile([128, 512], f32, tag="s")
                            nc.tensor.matmul(
                                ps_s[:, off:],
                                kt[h][kb // 4][:, (kb % 4) * 128:(kb % 4 + 1) * 128],
                                q_tiles[(h, qc)][:, off:],
                                start=True, stop=True)
                            if mi is not None:
                                msb = mask_sb[mi] if preload_masks else None
                                if msb is None:
                                    msb = mp.tile([128, 512], bf16, tag="mk",
                                                  name=f"mks{mi}")
                                    nc.gpsimd.dma_start(out=msb, in_=mask_d[mi, :, :])
                                # psum-read sbuf-write: in-place psum add
                                # would halve DVE rate (single psum port)
                                sm = smx_pool.tile([128, 512], f32, tag="sm")
                                nc.vector.tensor_add(sm[:, off:], ps_s[:, off:],
                                                     msb[:, off:])
                                exp_src = sm
                            else:
                                exp_src = ps_s
                            pr = probs_pool.tile([128, 512], bf16, tag="pr")
                            if off:
                                nc.gpsimd.memset(pr[:, 0:off], 0.0)
                            nc.scalar.activation(pr[:, off:], exp_src[:, off:],
                                                 Exp, bias=shiftb[:],
                                                 scale=float(SCALE))
                            if tri:
                                w = min(128, 512 - off)
                                nc.vector.tensor_mul(pr[:, off:off + w],
                                                     pr[:, off:off + w],
                                                     tri_sb[:, 0:w])
                            hst["prs"][j] = pr
                            return [(kb, pr, off)]

                        def emit_pair(h, hst, j1, j2):
                            kb1 = kbs[j1][0]
                            kb2 = kbs[j2][0]
                            ps2 = psB_s.tile([128, 2, 512], f32, tag="s2", bufs=1)
                            nc.tensor.matmul(
                                ps2[:, 0, :],
                                kt[h][kb1 // 4][:, (kb1 % 4) * 128:(kb1 % 4 + 1) * 128],
                                q_tiles[(h, qc)][:], start=True, stop=True)
                            nc.tensor.matmul(
                                ps2[:, 1, :],
                                kt[h][kb2 // 4][:, (kb2 % 4) * 128:(kb2 % 4 + 1) * 128],
                                q_tiles[(h, qc)][:], start=True, stop=True)
                            pr2 = probs_pool.tile([128, 2, 512], bf16, tag="pr2",
                                                  bufs=6)
                            nc.scalar.activation(pr2[:], ps2[:], Exp,
                                                 bias=shiftb[:],
                                                 scale=float(SCALE))
                            for idx, jj in ((0, j1), (1, j2)):
                                if kbs[jj][3]:
                                    nc.vector.tensor_mul(pr2[:, idx, 0:128],
                                                         pr2[:, idx, 0:128],
                                                         tri_sb[:, :])
                            hst["prs"][j1] = pr2[:, 0, :]
                            hst["prs"][j2] = pr2[:, 1, :]
                            return [(kb1, pr2[:, 0, :], 0), (kb2, pr2[:, 1, :], 0)]

                        def emit_pvs(h, hst, specs, last):
                            for si_, (kbp, prp, offp) in enumerate(specs):
                                nc.tensor.matmul(
                                    hst["att"][:, offp:],
                                    v_tiles[kbp][:, h * 128:(h + 1) * 128],
                                    prp[:, offp:],
                                    start=(hst["npv"] == 0),
                                    stop=(last and si_ == len(specs) - 1))
                                hst["npv"] += 1

                        for it_i, item in enumerate(sched):
                            for h in grp:
                                hst = st[h]
                                if item[0] == "s":
                                    specs = emit_single(h, hst, item[1])
                                else:
                                    specs = emit_pair(h, hst, *item[1])
                                # defer pv one item (and den two) so the next
                                # score matmuls keep PE fed during the exp
                                if hst["pend"] is not None:
                                    emit_pvs(h, hst, hst["pend"], False)
                                    if hst["pend2"] is not None:
                                        for (_, prp, _o) in hst["pend2"]:
                                            den_push(hst, prp)
                                hst["pend2"] = hst["pend"]
                                hst["pend"] = specs
                            if it_i in fjs:
                                emit_c_chunk(qc - 1, fjs[it_i])
                        for h in grp:
                            hst = st[h]
                            emit_pvs(h, hst, hst["pend"], True)
                            den_ps = psC.tile([128, 512], f32, tag="o",
                                              name=f"den{qc}_{h}")
                            if last33:
                                # last head: the final attn chain gates the
                                # closing o-proj chunks, so keep the DVE tree
                                # adds off it -- the last two blocks feed
                                # straight into accumulating den matmul
                                # streams; only exp/tri of block nkb-1 sits
                                # ahead of the den matmul on the chain
                                rem = [lv for lv in hst["levels"] if lv is not None]
                                cur = rem[0]
                                for lv in rem[1:]:
                                    s = dsum_pool.tile([128, 512], bf16, tag="ds")
                                    nc.vector.tensor_add(s[:], cur[:], lv[:])
                                    cur = s
                                nc.tensor.matmul(den_ps[:], onesq_r[:],
                                                 hst["prs"][nkb - 2][:],
                                                 start=True, stop=False)
                                nc.tensor.matmul(den_ps[:], onesq_r[:],
                                                 hst["prs"][nkb - 1][:],
                                                 start=False, stop=False)
                                nc.tensor.matmul(den_ps[:], onesq_r[:],
                                                 cur[:], start=False, stop=True)
                            else:
                                if hst["pend2"] is not None:
                                    for (_, prp, _o) in hst["pend2"]:
                                        den_push(hst, prp)
                                for (_, prp, _o) in hst["pend"]:
                                    den_push(hst, prp)
                                rem = [lv for lv in hst["levels"] if lv is not None]
                                cur = rem[0]
                                for lv in rem[1:]:
                                    s = dsum_pool.tile([128, 512], bf16, tag="ds")
                                    nc.vector.tensor_add(s[:], cur[:], lv[:])
                                    cur = s
                                nc.tensor.matmul(den_ps[:], onesq_r[:],
                                                 cur[:], start=True, stop=True)
                            # den evict frees the den psum slot for the next
                            # user; the rest of the tail is deferred one head
                            # for pipelining
                            if last33:
                                # last tail: reciprocal straight from PSUM;
                                # skipping the copy shortens the final attn
                                # chain gating the closing o-proj chunks
                                tails.append((h, qc, hst["att"], den_ps))
                            else:
                                den_sb = den_pool.tile([128, 512], f32, tag="dn")
                                nc.vector.tensor_copy(out=den_sb[:], in_=den_ps[:])
                                tails.append((h, qc, hst["att"], den_sb))
                        if post_fill:
                            emit_c_chunk(qc - 1, post_fill)
                    if qc > 0:
                        for hh in range(HPC):
                            del attn_tiles[(hh, qc - 1)]
                while tails:
                    emit_tail(tails.pop(0))
                emit_c_chunk(3, range(16))
                for hh in range(HPC):
                    del attn_tiles[(hh, 3)]

    nc.compile()
    return nc


LAST_EXEC_NS = None


def kernel(hidden_states, Wq, Wk, Wv, Wo, attention_mask):
    global LAST_EXEC_NS
    from concourse.bass_utils import run_bass_kernel_spmd

    hidden_states = np.asarray(hidden_states, dtype=np.float32)
    Wq = np.asarray(Wq, dtype=np.float32)
    Wk = np.asarray(Wk, dtype=np.float32)
    Wv = np.asarray(Wv, dtype=np.float32)
    Wo = np.asarray(Wo, dtype=np.float32)
    attention_mask = np.asarray(attention_mask, dtype=np.float32)

    cosT, ssinT = _rope_tables()
    plan, mtiles = _mask_plan(attention_mask[0])
    nm = len(mtiles)
    preload = nm <= 24
    any_tri = any(e[3] for row in plan for e in row)
    maskt = np.stack(mtiles).astype(BF16) if nm else None

    plan_key = (tuple(tuple(r) for r in plan), nm, preload, any_tri)
    nc = _PROGRAM_CACHE.get(plan_key)
    if nc is None:
        nc = _build_program(plan, nm, preload, any_tri)
        _PROGRAM_CACHE[plan_key] = nc

    perm = np.concatenate([np.arange(0, HD, 2), np.arange(1, HD, 2)])
    Wq4 = Wq.reshape(NH, HD, HID)[:, perm, :]
    Wk4 = Wk.reshape(NH, HD, HID)[:, perm, :]
    Wv4 = Wv.reshape(NH, HD, HID)

    # [4, 128, 16, 512] per-partition-contiguous hs tiling, per batch (bf16)
    hs_tl = [np.ascontiguousarray(
        hidden_states[b].reshape(4, 512, 16, 128).transpose(0, 3, 2, 1)).astype(BF16)
        for b in range(B)]

    def tile_qk(mT):   # [HID, 512] -> [128, 16, 4, 128]
        return np.ascontiguousarray(
            mT.reshape(16, 128, 4, 128).transpose(1, 0, 2, 3)).astype(BF16)

    tri_t = np.triu(np.ones((128, 128), dtype=np.float32)).astype(BF16)

    in_maps = []
    for c in range(NCORES):
        b, hg = divmod(c, HPC)
        heads = slice(hg * HPC, (hg + 1) * HPC)
        wqT = Wq4[heads].reshape(512, HID).T          # [HID, 512]
        wkT = Wk4[heads].reshape(512, HID).T
        wvT = Wv4[heads].reshape(512, HID).T          # [HID, 512]
        woT = Wo[:, hg * 512:(hg + 1) * 512].T        # [512, HID]
        m = {
            "hs_t": hs_tl[b],
            "wq_t": tile_qk(wqT),
            "wk_t": tile_qk(wkT),
            "wv_t": np.ascontiguousarray(
                wvT.reshape(16, 128, 512).transpose(1, 0, 2)).astype(BF16),
            "wo_t": np.ascontiguousarray(
                woT.reshape(4, 128, 16, 128).transpose(1, 0, 2, 3)).astype(BF16),
            "cosT": cosT.astype(BF16),
            "ssinT": ssinT.astype(BF16),
        }
        if any_tri:
            m["tri_t"] = tri_t
        if nm:
            m["maskt"] = maskt
        in_maps.append(m)

    trace = bool(os.environ.get("CC_BASS_TRACE"))
    res = run_bass_kernel_spmd(nc, in_maps, core_ids=list(range(NCORES)), trace=trace)
    LAST_EXEC_NS = res.exec_time_ns

    out = np.empty((B, S, S), dtype=np.float32)
    for b in range(B):
        acc = res.results[b * HPC]["oT_t"].astype(np.float32)
        for hg in range(1, HPC):
            acc = acc + res.results[b * HPC + hg]["oT_t"].astype(np.float32)
        # [qc, iblk, p, t] -> [iblk*128+p, qc*512+t] = oT_full, out = oT_full.T
        o_full = acc.transpose(1, 2, 0, 3).reshape(S, S)
        out[b] = o_full.T
    return out


# revision 29
# speedup vs baseline: 1.0415x; 1.0415x over previous
"""Evo2 attention (B=2, S=2048, HID=2048, NH=16, HD=128) on 8 trn2 NeuronCores.

Sharding: core c handles batch b=c//4 and heads 4*(c%4)..4*(c%4)+3.
Megatron-style: q/k/v projections column-parallel, o_proj row-parallel with the
4-way partial sum done on host during unshard.

Per-core kernel layout (everything transposed so no on-chip transposes needed):
  hsT [hid, tok] -> qT,kT [hd, tok] (RoPE fused into PSUM eviction, rotate-half
  basis obtained by de-interleaving W rows on host), v [tok, hd].
  scoresT[k, q] = kT_blk vs qT matmul; softmax over k (= partitions) with a
  fixed shift instead of a max; denominators via ones-vector PE reduction and
  a K=1 matmul broadcast; PV gives attnT [hd, q]; o_projT partial [o, q].

Optimizations vs the fp32r baseline (429us -> 331us -> ~318us):
  - all matmul operands bf16 (PSUM accumulation stays fp32): halves DMA+SBUF,
    enables FWL weight loads; output partials bf16, summed fp32 on host
  - phase A head kill: q and k passes are kc-OUTER across 4 psum banks each
    with 2-kc weight piece tiles (2KB DMA lines -- 1-kc pieces halve the DMA
    rate) and consumption-ordered DMA interleave across BOTH rings (hs on
    sync, wq on gpsimd), so the first real matmul gates on ~768KB (not 4MB)
    and chunk 0 streams DMA-paced from ~10us. Warmup = many SMALL dummy
    matmuls (N=128/64): the preamble end varies +-2.5us run to run, so when
    data arrives mid-warmup the queued dummies must drain in ~100ns steps.
  - early dummy exp (ACT table-set preload off the critical path)
  - causal diagonal blocks column-trimmed: fully-masked leading q-columns of
    each [128k x 512q] tile are never computed (score/exp/den/PV all skip)
  - causal masking via a [128,128] bf16 triangle MULTIPLY on probs post-exp
    (masked probs become exact zeros) instead of a [128,512] f32 mask add
    pre-exp: ~25us less DVE work and a shorter score->exp chain. Generic
    (non-causal-triangle) masks keep the old add-a-mask-tile path.
  - reciprocal -> single-pass reciprocal_approx_fast (~5x faster DVE op)
  - denominators: off>0 probs tiles zero-padded by gpsimd memsets so every
    (h,qc) collapses via a rolling binary tree of bf16 DVE adds into ONE
    N=512 PE reduction stream (den matmuls were ~11us of PE before); for the
    last head the final two blocks feed accumulating den matmuls directly so
    no DVE adds sit on the closing attn chain. (Routing the level-0 adds to
    gpsimd tensor_add was tried and cost +40us -- gpsimd elementwise is far
    slower than its memsets; don't.)
  - qc=0 (no o-proj filler available) runs its heads as interleaved PAIRS
    for chain-level parallelism; elsewhere o-projection of the previous
    q-chunk is interleaved INTO the score/PV loop at block granularity as PE
    filler for the exp-chain waits, with 2 iblks held back past the last
    head's j-loop to cover its den/reciprocal/attn drain
  - dependency-granularity fixes: per-(head,chunk) kt tiles, perm-scoped mask
    pool
"""
import os
import sys
import numpy as np

for _p in ("/opt/trn_rl_repo",):
    if os.path.isdir(_p) and _p not in sys.path:
        sys.path.insert(0, _p)

import ml_dtypes

BF16 = ml_dtypes.bfloat16
B, S, HID, NH = 2, 2048, 2048, 16
HD = HID // NH            # 128
HPC = 4                   # heads per core
NCORES = 8
BASE = 10000.0
SCALE = 1.0 / np.sqrt(HD).astype(np.float32)
SHIFT = 25.0              # fixed softmax shift (replaces per-row max)
NEG_INF_THRESH = -1e8

_PROGRAM_CACHE = {}


def _rope_tables():
    """cos/±sin tables [HD, S] in the de-interleaved (rotate-half) basis.

    Reference pairs dims (2m, 2m+1) with angle theta_m(s) = s * inv_freq[f(m)],
    f(m) = 2m for m<32 else 2m-64 (from emb[:, ::2] of concat([freqs, freqs])).
    After de-interleave perm [0,2,..126,1,3,..127]: new dim m<64 is old 2m,
    new dim 64+m is old 2m+1.
      out[m]    = x[m] cos_m - x[64+m] sin_m
      out[64+m] = x[m] sin_m + x[64+m] cos_m
    """
    inv_freq = BASE ** (-np.arange(0, HD, 2, dtype=np.float64) / HD)  # [64]
    m = np.arange(64)
    fmap = np.where(m < 32, 2 * m, 2 * m - 64)
    t = np.arange(S, dtype=np.float64)
    theta = t[None, :] * inv_freq[fmap][:, None]          # [64, S]
    cos = np.cos(theta)
    sin = np.sin(theta)
    cosT = np.concatenate([cos, cos], axis=0).astype(np.float32)      # [128, S]
    # row d holds the factor applied to SOURCE half d (dest = other half):
    # src lo -> dst hi uses +sin; src hi -> dst lo uses -sin
    ssinT = np.concatenate([sin, -sin], axis=0).astype(np.float32)    # [128, S]
    return cosT, ssinT


def _mask_plan(mask2d):
    """Classify [128k x 512q] blocks of mask^T. Returns (plan, tiles).

    plan[qc] = list of (kb, mask_tile_idx_or_None, col_off, tri); fully-masked
    blocks skipped; col_off = count of leading fully-masked q-columns (those
    columns are skipped entirely in score/exp/den/PV). tri=True means the
    block's only masking is a causal triangle in its leading 128 columns
    (after the off trim) -- handled by a post-exp triangle multiply, no mask
    tile needed. tiles: deduped f32 [128, 512] mask^T blocks prescaled by
    sqrt(HD) for the generic fallback path.
    """
    maskT = np.ascontiguousarray(mask2d.T)  # [k, q]
    plan = []
    tiles = []
    seen = {}
    for qc in range(S // 512):
        row = []
        for kb in range(S // 128):
            sub = maskT[kb * 128:(kb + 1) * 128, qc * 512:(qc + 1) * 512]
            masked = sub <= NEG_INF_THRESH
            if masked.all():
                continue
            colmasked = masked.all(axis=0)  # [512]
            off = 0
            while off < 512 and colmasked[off]:
                off += 1
            if (sub[:, off:] == 0.0).all():
                row.append((kb, None, off, False))
                continue
            w = min(128, 512 - off)
            tri_pat = np.tril(np.ones((128, 128), dtype=bool), -1)[:, :w]
            sub_w = sub[:, off:off + w]
            if (np.array_equal(masked[:, off:off + w], tri_pat)
                    and (sub_w[~tri_pat] == 0.0).all()
                    and (sub[:, off + w:] == 0.0).all()):
                row.append((kb, None, off, True))
                continue
            pre = np.ascontiguousarray(sub * np.float32(np.sqrt(HD)))
            key = pre.tobytes()
            idx = seen.get(key)
            if idx is None:
                idx = len(tiles)
                seen[key] = idx
                tiles.append(pre)
            row.append((kb, idx, off, False))
        plan.append(row)
    return plan, tiles


def _build_program(plan, nm, preload_masks, any_tri):
    import contextlib
    import concourse.bacc as bacc
    import concourse.tile as tile
    from concourse import mybir

    f32 = mybir.dt.float32
    f32r = mybir.dt.float32r
    bf16 = mybir.dt.bfloat16
    nc = bacc.Bacc(None, target_bir_lowering=False)

    # host-pre-tiled inputs: last axis group per partition is contiguous
    hs_d = nc.dram_tensor("hs_t", [4, 128, 16, 512], bf16, kind="ExternalInput")
    wq_d = nc.dram_tensor("wq_t", [128, 16, 4, 128], bf16, kind="ExternalInput")
    wk_d = nc.dram_tensor("wk_t", [128, 16, 4, 128], bf16, kind="ExternalInput")
    wv_d = nc.dram_tensor("wv_t", [128, 16, 512], bf16, kind="ExternalInput")
    wo_d = nc.dram_tensor("wo_t", [128, 4, 16, 128], bf16, kind="ExternalInput")
    cos_d = nc.dram_tensor("cosT", [128, S], bf16, kind="ExternalInput")
    sin_d = nc.dram_tensor("ssinT", [128, S], bf16, kind="ExternalInput")
    if any_tri:
        tri_d = nc.dram_tensor("tri_t", [128, 128], bf16, kind="ExternalInput")
    if nm:
        mask_d = nc.dram_tensor("maskt", [nm, 128, 512], bf16, kind="ExternalInput")
    o_d = nc.dram_tensor("oT_t", [4, 16, 128, 512], bf16, kind="ExternalOutput")

    Exp = mybir.ActivationFunctionType.Exp

    with tile.TileContext(nc) as tc:
        with contextlib.ExitStack() as perm:
            kt_pool = perm.enter_context(tc.tile_pool(name="kt", bufs=16))
            qa_pool = perm.enter_context(tc.tile_pool(name="qa", bufs=17))
            cst = perm.enter_context(tc.tile_pool(name="cst", bufs=1))
            tmp_pool = perm.enter_context(tc.tile_pool(name="tmp", bufs=2))
            wo_pool = perm.enter_context(tc.tile_pool(name="wo", bufs=1))

            wup_st = cst.tile([128, 512], f32, tag="wu0")
            nc.vector.memset(wup_st, 0.0)
            wup = cst.tile([128, 512], bf16, tag="wu1")
            nc.vector.tensor_copy(out=wup, in_=wup_st)

            onesq_st = cst.tile([128, 128], f32, tag="o3")
            nc.vector.memset(onesq_st, 1.0)
            onesq_r = cst.tile([128, 128], bf16, tag="o4")
            nc.vector.tensor_copy(out=onesq_r, in_=onesq_st)
            ones_r1 = cst.tile([128, 128], f32r, tag="o5")
            nc.vector.tensor_copy(out=ones_r1[0:1, :], in_=onesq_st[0:1, :])
            shiftb = cst.tile([128, 1], f32, tag="sh")
            nc.vector.memset(shiftb, -SHIFT)
            # preload the exp ACT table set now; otherwise the first phase-B
            # exp pays the ~2.7us ACT_TABLE_LOAD + DRAIN on the critical path
            expwarm = cst.tile([128, 1], f32, tag="ew")
            nc.scalar.activation(expwarm[:], shiftb[:], Exp)

            tri_sb = None
            if any_tri:
                tri_sb = cst.tile([128, 128], bf16, tag="tri")
                nc.gpsimd.dma_start(out=tri_sb, in_=tri_d[:, :])

            # per-(head, chunk) kt tiles: tile-granular dependency tracking
            # means one big [128,S] tile would gate phase B's first score
            # matmul on the LAST chunk's rope writes
            kt = [[kt_pool.tile([128, 512], bf16, tag="kt", name=f"kt{i}_{c}")
                   for c in range(4)] for i in range(HPC)]

            # mask tiles live in a perm-scoped pool: allocated inside phase
            # B's stack their SBUF range overlaps phase A's pools, which makes
            # the DMA wait for ALL phase-A matmuls and puts ~5us of mask
            # latency on the first exp chain.
            mask_sb = None
            if nm and preload_masks:
                mkp = perm.enter_context(tc.tile_pool(name="mk", bufs=nm))
                mask_sb = [mkp.tile([128, 512], bf16, tag="mk", name=f"mk{i}")
                           for i in range(nm)]
            v_tiles = [None] * 16
            q_tiles = {}
            attn_tiles = {}

            def rope_evict(ps, dst, cos_sb, sin_sb):
                nc.vector.tensor_mul(dst[0:64, :], ps[64:128, :], sin_sb[64:128, :])
                nc.vector.tensor_mul(dst[64:128, :], ps[0:64, :], sin_sb[0:64, :])
                t = tmp_pool.tile([128, 512], f32, tag="ropetmp")
                nc.vector.tensor_mul(t[:], ps[:], cos_sb[:, :])
                nc.vector.tensor_add(dst[:, :], dst[:, :], t[:])

            # ---- Phase A: projections. One hs residency per 512-token
            # chunk feeds the q, k AND v passes. q and k passes are kc-outer
            # (4 parallel psum accumulation groups) so the instruction stream
            # consumes weight pieces in DMA arrival order -- chunk 0 runs
            # DMA-paced with no dead head.
            with contextlib.ExitStack() as actx:
                hs_pool = actx.enter_context(tc.tile_pool(name="hs", bufs=8))
                tabp = actx.enter_context(tc.tile_pool(name="tab", bufs=4))
                w_pool = actx.enter_context(tc.tile_pool(name="w", bufs=1))
                wqp_pool = actx.enter_context(tc.tile_pool(name="wqp", bufs=8))
                wkp_pool = actx.enter_context(tc.tile_pool(name="wkp", bufs=8))
                psQ = actx.enter_context(tc.tile_pool(name="psQ", bufs=4, space="PSUM"))
                psK = actx.enter_context(tc.tile_pool(name="psK", bufs=4, space="PSUM"))
                v_pool = perm.enter_context(tc.tile_pool(name="v", bufs=16, side="right"))

                def load_hs(c):
                    ts = []
                    for i in range(4):
                        ht = hs_pool.tile([128, 4, 512], bf16, tag="hs",
                                          name=f"hs{c}_{i}")
                        nc.sync.dma_start(out=ht, in_=hs_d[c, :, 4 * i:4 * i + 4, :])
                        ts.append(ht)
                    return ts

                # ~5us of SMALL dummy matmuls: HAM un-throttles while the
                # first DMA pieces land. Small N so that when data arrives
                # mid-warmup, the queued dummies drain in ~100ns steps (the
                # preamble end time varies +-2.5us run to run, so the warmup
                # cannot be tuned to a fixed matmul count at N=512).
                ps_wu = psK.tile([128, 512], f32, tag="k", name="pswu")
                for i in range(40):
                    nc.tensor.matmul(ps_wu[:, 0:128], wup[:, 0:128],
                                     wup[:, 0:128], start=True, stop=True)
                for i in range(24):
                    nc.tensor.matmul(ps_wu[:, 0:64], wup[:, 0:128],
                                     wup[:, 0:64], start=True, stop=True)

                wv_all = w_pool.tile([128, 16, 512], bf16, tag="wv", name="wvall")
                wo_all = wo_pool.tile([128, 4, 16, 128], bf16, tag="wo", name="woall")
                # consumption-ordered interleave: the kc-outer q-pass needs
                # only (hs piece kc//4, wq piece kc//2) per kc step, so the
                # first real matmul gates on ~768KB and the pass streams
                # DMA-paced. Pieces are 2-kc (weights) / 4-kc (hs) so the
                # per-partition DMA lines stay 2KB/4KB (1-kc pieces halve the
                # achieved DMA rate).
                wqp = [wqp_pool.tile([128, 2, 4, 128], bf16, tag="wqp",
                                     name=f"wqp{p}") for p in range(8)]
                wkp = [wkp_pool.tile([128, 2, 4, 128], bf16, tag="wkp",
                                     name=f"wkp{p}") for p in range(8)]
                # wq pieces go on the gpsimd DMA ring: both rings fill in
                # parallel, so the first real matmul's gate (hs piece 0 +
                # wq piece 0) lands ~2us sooner than on one serial ring
                hs_cur = []
                for i in range(4):
                    ht = hs_pool.tile([128, 4, 512], bf16, tag="hs", name=f"hs0_{i}")
                    nc.sync.dma_start(out=ht, in_=hs_d[0, :, 4 * i:4 * i + 4, :])
                    hs_cur.append(ht)
                    nc.gpsimd.dma_start(out=wqp[2 * i], in_=wq_d[:, 4 * i:4 * i + 2, :, :])
                    nc.gpsimd.dma_start(out=wqp[2 * i + 1], in_=wq_d[:, 4 * i + 2:4 * i + 4, :, :])
                for p in range(8):
                    nc.sync.dma_start(out=wkp[p], in_=wk_d[:, 2 * p:2 * p + 2, :, :])
                for p in range(4):
                    nc.sync.dma_start(out=wv_all[:, p * 4:(p + 1) * 4, :],
                                      in_=wv_d[:, p * 4:(p + 1) * 4, :])
                for p in range(4):
                    nc.sync.dma_start(out=wo_all[:, p, :, :],
                                      in_=wo_d[:, p, :, :])
                if mask_sb is not None:
                    for i in range(nm):
                        nc.sync.dma_start(out=mask_sb[i], in_=mask_d[i, :, :])
                for c in range(4):
                    hs_nxt = load_hs(c + 1) if c < 3 else None
                    cos_sb = tabp.tile([128, 512], bf16, tag="cos")
                    nc.gpsimd.dma_start(out=cos_sb, in_=cos_d[:, c * 512:(c + 1) * 512])
                    sin_sb = tabp.tile([128, 512], bf16, tag="sin")
                    nc.gpsimd.dma_start(out=sin_sb, in_=sin_d[:, c * 512:(c + 1) * 512])
                    # q-pass, kc-outer across 4 psum banks
                    psq = [psQ.tile([128, 512], f32, tag="q", name=f"psq{c}_{d}")
                           for d in range(4)]
                    for kc in range(16):
                        for dblk in range(4):
                            nc.tensor.matmul(
                                psq[dblk][:], wqp[kc // 2][:, kc % 2, dblk, :],
                                hs_cur[kc // 4][:, kc % 4, :],
                                start=(kc == 0), stop=(kc == 15))
                    for dblk in range(4):
                        q = qa_pool.tile([128, 512], bf16, tag="qa")
                        rope_evict(psq[dblk], q, cos_sb, sin_sb)
                        q_tiles[(dblk, c)] = q
                    # k-pass, kc-outer (separate banks: no wait on q evicts)
                    psk = [psK.tile([128, 512], f32, tag="k", name=f"psk{c}_{d}")
                           for d in range(4)]
                    for kc in range(16):
                        for dblk in range(4):
                            nc.tensor.matmul(
                                psk[dblk][:], wkp[kc // 2][:, kc % 2, dblk, :],
                                hs_cur[kc // 4][:, kc % 4, :],
                                start=(kc == 0), stop=(kc == 15))
                    for dblk in range(4):
                        rope_evict(psk[dblk], kt[dblk][c], cos_sb, sin_sb)
                    # v pass (vblk-outer, kc-inner; reuses the q psum banks
                    # after their rope evictions, evictions stagger onto ACT)
                    for vblk in range(4):
                        psv = psQ.tile([128, 512], f32, tag="q",
                                       name=f"psv{c}_{vblk}")
                        for kc in range(16):
                            nc.tensor.matmul(
                                psv[:],
                                hs_cur[kc // 4][:, kc % 4, vblk * 128:(vblk + 1) * 128],
                                wv_all[:, kc, :], start=(kc == 0), stop=(kc == 15))
                        vt = v_pool.tile([128, 512], bf16, tag="v", name=f"v{c}_{vblk}")
                        nc.scalar.copy(out=vt[:], in_=psv[:])
                        v_tiles[c * 4 + vblk] = vt
                    hs_cur = hs_nxt

            # ---------------- Phase B + C ------------------------------------
            with contextlib.ExitStack() as bctx:
                probs_pool = bctx.enter_context(tc.tile_pool(name="pr", bufs=12))
                dsum_pool = bctx.enter_context(tc.tile_pool(name="ds", bufs=12))
                smx_pool = bctx.enter_context(tc.tile_pool(name="sm", bufs=2))
                den_pool = bctx.enter_context(tc.tile_pool(name="dn", bufs=4))
                rcb_pool = bctx.enter_context(tc.tile_pool(name="rcb", bufs=3))
                outb_pool = bctx.enter_context(tc.tile_pool(name="ob", bufs=6))
                psB_s = bctx.enter_context(tc.tile_pool(name="psBs", bufs=3, space="PSUM"))
                psB_a = bctx.enter_context(tc.tile_pool(name="psBa", bufs=2, space="PSUM"))
                psB_d = bctx.enter_context(tc.tile_pool(name="psBd", bufs=1, space="PSUM"))
                psC = bctx.enter_context(tc.tile_pool(name="psC", bufs=2, space="PSUM"))
                if nm and not preload_masks:
                    mp = bctx.enter_context(tc.tile_pool(name="mk", bufs=8))

                def emit_tail(state):
                    h, qc, ps_att, den_sb = state
                    rcb = rcb_pool.tile([128, 512], f32, tag="rcb")
                    nc.vector.reciprocal_approx_fast(out=rcb[:], in_=den_sb[:])
                    at = qa_pool.tile([128, 512], bf16, tag="qa")
                    nc.vector.tensor_mul(at[:], ps_att[:], rcb[:])
                    attn_tiles[(h, qc)] = at

                def emit_c_chunk(qc, iblks):
                    for iblk in iblks:
                        ps_o = psC.tile([128, 512], f32, tag="o")
                        for jc in range(4):
                            nc.tensor.matmul(ps_o[:], wo_all[:, jc, iblk, :],
                                             attn_tiles[(jc, qc)][:],
                                             start=(jc == 0), stop=(jc == 3))
                        ob = outb_pool.tile([128, 512], bf16, tag="ob")
                        if iblk % 8 < 6:
                            nc.vector.tensor_copy(out=ob[:], in_=ps_o[:])
                        else:
                            nc.scalar.copy(out=ob[:], in_=ps_o[:])
                        nc.sync.dma_start(out=o_d[qc, iblk, :, :], in_=ob[:])

                def den_push(hst, t):
                    # den: every probs tile is full-width valid (off>0 blocks
                    # are zero-padded by an idle-engine gpsimd memset), so the
                    # whole (h,qc) collapses via a rolling binary tree of bf16
                    # DVE adds into ONE N=512 PE reduction stream (pushed 2
                    # blocks behind the score loop, off the PE critical path)
                    levels = hst["levels"]
                    lvl = 0
                    cur = t
                    while levels[lvl] is not None:
                        s = dsum_pool.tile([128, 512], bf16, tag="ds")
                        nc.vector.tensor_add(s[:], levels[lvl][:], cur[:])
                        levels[lvl] = None
                        cur = s
                        lvl += 1
                    levels[lvl] = cur

                tails = []
                for qc in range(4):
                    kbs = plan[qc]
                    nkb = len(kbs)
                    # qc=0 has no o-proj filler available (no previous qc), so
                    # its short 4-block chains run as interleaved HEAD PAIRS
                    # for chain-level parallelism instead
                    groups = [(0, 1), (2, 3)] if qc == 0 else \
                             [(h,) for h in range(HPC)]
                    for grp in groups:
                        # flush deferred tails first: the in-loop o-proj
                        # filler needs the previous qc's LAST attn tile
                        while tails:
                            emit_tail(tails.pop(0))
                        st = {}
                        for h in grp:
                            st[h] = {
                                "att": psB_a.tile([128, 512], f32, tag="att",
                                                  name=f"att{qc}_{h}"),
                                "den": psB_d.tile([128, 512], f32, tag="d",
                                                  name=f"den{qc}_{h}"),
                                "prs": [None] * nkb,
                                "levels": [None] * 6,
                                "pend": None,
                            }
                        # o-proj filler positions: prev-qc iblks emitted inside
                        # the j loop (phase B is exp-chain paced; the
                        # independent o-proj matmuls absorb the PE waits). For
                        # the very last head, 2 iblks are held back to fill
                        # the PE gap while its den/tail chain drains.
                        post_fill = []
                        fjs = {}
                        if qc > 0:
                            h = grp[0]
                            base = 4 * h
                            if qc == 3 and h == 3:
                                fjs = {nkb // 2 - 1: [base], nkb - 1: [base + 1]}
                                post_fill = [base + 2, base + 3]
                            else:
                                fjs = {(i + 1) * nkb // 4 - 1: [base + i]
                                       for i in range(4)}
                        for j, (kb, mi, off, tri) in enumerate(kbs):
                            for h in grp:
                                hst = st[h]
                                ps_s = psB_s.tile([128, 512], f32, tag="s")
                                nc.tensor.matmul(
                                    ps_s[:, off:],
                                    kt[h][kb // 4][:, (kb % 4) * 128:(kb % 4 + 1) * 128],
                                    q_tiles[(h, qc)][:, off:],
                                    start=True, stop=True)
                                if mi is not None:
                                    msb = mask_sb[mi] if preload_masks else None
                                    if msb is None:
                                        msb = mp.tile([128, 512], bf16, tag="mk",
                                                      name=f"mks{mi}")
                                        nc.gpsimd.dma_start(out=msb, in_=mask_d[mi, :, :])
                                    # psum-read sbuf-write: in-place psum add
                                    # would halve DVE rate (single psum port)
                                    sm = smx_pool.tile([128, 512], f32, tag="sm")
                                    nc.vector.tensor_add(sm[:, off:], ps_s[:, off:],
                                                         msb[:, off:])
                                    exp_src = sm
                                else:
                                    exp_src = ps_s
                                pr = probs_pool.tile([128, 512], bf16, tag="pr")
                                if off:
                                    nc.gpsimd.memset(pr[:, 0:off], 0.0)
                                nc.scalar.activation(pr[:, off:], exp_src[:, off:],
                                                     Exp, bias=shiftb[:],
                                                     scale=float(SCALE))
                                if tri:
                                    w = min(128, 512 - off)
                                    nc.vector.tensor_mul(pr[:, off:off + w],
                                                         pr[:, off:off + w],
                                                         tri_sb[:, 0:w])
                                hst["prs"][j] = pr
                                # defer pv one kb (and den two) so the next
                                # score matmul keeps PE fed during exp(j)
                                if hst["pend"] is not None:
                                    jp, kbp, prp, offp = hst["pend"]
                                    nc.tensor.matmul(
                                        hst["att"][:, offp:],
                                        v_tiles[kbp][:, h * 128:(h + 1) * 128],
                                        prp[:, offp:],
                                        start=(jp == 0), stop=False)
                                    if jp >= 1:
                                        den_push(hst, hst["prs"][jp - 1])
                                hst["pend"] = (j, kb, pr, off)
                            if j in fjs:
                                emit_c_chunk(qc - 1, fjs[j])
                        for h in grp:
                            hst = st[h]
                            jp, kbp, prp, offp = hst["pend"]
                            nc.tensor.matmul(hst["att"][:, offp:],
                                             v_tiles[kbp][:, h * 128:(h + 1) * 128],
                                             prp[:, offp:],
                                             start=(jp == 0), stop=True)
                            if qc == 3 and h == HPC - 1:
                                # last head: the final attn chain gates the
                                # closing o-proj chunks, so keep the DVE tree
                                # adds off it -- the last two blocks feed
                                # straight into accumulating den matmul
                                # streams; only exp/tri of block nkb-1 sits
                                # ahead of the den matmul on the chain
                                rem = [lv for lv in hst["levels"] if lv is not None]
                                cur = rem[0]
                                for lv in rem[1:]:
                                    s = dsum_pool.tile([128, 512], bf16, tag="ds")
                                    nc.vector.tensor_add(s[:], cur[:], lv[:])
                                    cur = s
                                nc.tensor.matmul(hst["den"][:], onesq_r[:],
                                                 hst["prs"][nkb - 2][:],
                                                 start=True, stop=False)
                                nc.tensor.matmul(hst["den"][:], onesq_r[:],
                                                 hst["prs"][nkb - 1][:],
                                                 start=False, stop=False)
                                nc.tensor.matmul(hst["den"][:], onesq_r[:],
                                                 cur[:], start=False, stop=True)
                            else:
                                if jp >= 1:
                                    den_push(hst, hst["prs"][jp - 1])
                                den_push(hst, hst["prs"][nkb - 1])
                                rem = [lv for lv in hst["levels"] if lv is not None]
                                cur = rem[0]
                                for lv in rem[1:]:
                                    s = dsum_pool.tile([128, 512], bf16, tag="ds")
                                    nc.vector.tensor_add(s[:], cur[:], lv[:])
                                    cur = s
                                nc.tensor.matmul(hst["den"][:], onesq_r[:],
                                                 cur[:], start=True, stop=True)
                            # den evict frees the den psum bank for the next
                            # head; the rest of the tail is deferred one head
                            # for pipelining
                            if qc == 3 and h == HPC - 1:
                                # last tail: reciprocal straight from PSUM;
                                # skipping the copy shortens the final attn
                                # chain gating the closing o-proj chunks
                                tails.append((h, qc, hst["att"], hst["den"]))
                            else:
                                den_sb = den_pool.tile([128, 512], f32, tag="dn")
                                nc.vector.tensor_copy(out=den_sb[:], in_=hst["den"][:])
                                tails.append((h, qc, hst["att"], den_sb))
                        if post_fill:
                            emit_c_chunk(qc - 1, post_fill)
                    if qc > 0:
                        for hh in range(HPC):
                            del attn_tiles[(hh, qc - 1)]
                while tails:
                    emit_tail(tails.pop(0))
                emit_c_chunk(3, range(16))
                for hh in range(HPC):
                    del attn_tiles[(hh, 3)]

    nc.compile()
    return nc


LAST_EXEC_NS = None


def kernel(hidden_states, Wq, Wk, Wv, Wo, attention_mask):
    global LAST_EXEC_NS
    from concourse.bass_utils import run_bass_kernel_spmd

    hidden_states = np.asarray(hidden_states, dtype=np.float32)
    Wq = np.asarray(Wq, dtype=np.float32)
    Wk = np.asarray(Wk, dtype=np.float32)
    Wv = np.asarray(Wv, dtype=np.float32)
    Wo = np.asarray(Wo, dtype=np.float32)
    attention_mask = np.asarray(attention_mask, dtype=np.float32)

    cosT, ssinT = _rope_tables()
    plan, mtiles = _mask_plan(attention_mask[0])
    nm = len(mtiles)
    preload = nm <= 24
    any_tri = any(e[3] for row in plan for e in row)
    maskt = np.stack(mtiles).astype(BF16) if nm else None

    plan_key = (tuple(tuple(r) for r in plan), nm, preload, any_tri)
    nc = _PROGRAM_CACHE.get(plan_key)
    if nc is None:
        nc = _build_program(plan, nm, preload, any_tri)
        _PROGRAM_CACHE[plan_key] = nc

    perm = np.concatenate([np.arange(0, HD, 2), np.arange(1, HD, 2)])
    Wq4 = Wq.reshape(NH, HD, HID)[:, perm, :]
    Wk4 = Wk.reshape(NH, HD, HID)[:, perm, :]
    Wv4 = Wv.reshape(NH, HD, HID)

    # [4, 128, 16, 512] per-partition-contiguous hs tiling, per batch (bf16)
    hs_tl = [np.ascontiguousarray(
        hidden_states[b].reshape(4, 512, 16, 128).transpose(0, 3, 2, 1)).astype(BF16)
        for b in range(B)]

    def tile_qk(mT):   # [HID, 512] -> [128, 16, 4, 128]
        return np.ascontiguousarray(
            mT.reshape(16, 128, 4, 128).transpose(1, 0, 2, 3)).astype(BF16)

    tri_t = np.triu(np.ones((128, 128), dtype=np.float32)).astype(BF16)

    in_maps = []
    for c in range(NCORES):
        b, hg = divmod(c, HPC)
        heads = slice(hg * HPC, (hg + 1) * HPC)
        wqT = Wq4[heads].reshape(512, HID).T          # [HID, 512]
        wkT = Wk4[heads].reshape(512, HID).T
        wvT = Wv4[heads].reshape(512, HID).T          # [HID, 512]
        woT = Wo[:, hg * 512:(hg + 1) * 512].T        # [512, HID]
        m = {
            "hs_t": hs_tl[b],
            "wq_t": tile_qk(wqT),
            "wk_t": tile_qk(wkT),
            "wv_t": np.ascontiguousarray(
                wvT.reshape(16, 128, 512).transpose(1, 0, 2)).astype(BF16),
            "wo_t": np.ascontiguousarray(
                woT.reshape(4, 128, 16, 128).transpose(1, 0, 2, 3)).astype(BF16),
            "cosT": cosT.astype(BF16),
            "ssinT": ssinT.astype(BF16),
        }
        if any_tri:
            m["tri_t"] = tri_t
        if nm:
            m["maskt"] = maskt
        in_maps.append(m)

    trace = bool(os.environ.get("CC_BASS_TRACE"))
    res = run_bass_kernel_spmd(nc, in_maps, core_ids=list(range(NCORES)), trace=trace)
    LAST_EXEC_NS = res.exec_time_ns

    out = np.empty((B, S, S), dtype=np.float32)
    for b in range(B):
        acc = res.results[b * HPC]["oT_t"].astype(np.float32)
        for hg in range(1, HPC):
            acc = acc + res.results[b * HPC + hg]["oT_t"].astype(np.float32)
        # [qc, iblk, p, t] -> [iblk*128+p, qc*512+t] = oT_full, out = oT_full.T
        o_full = acc.transpose(1, 2, 0, 3).reshape(S, S)
        out[b] = o_full.T
    return out
